# revision 22
# baseline (speedup 1.0000x reference)
"""Trainium2 Bass kernel for nn_ExpertModule (dense transformer block).

Strategy: data-parallel over batch across 8 NeuronCores (16 batches -> 2 per
core), small parameter set replicated to every core.  All heavy compute stays
on-chip per batch element (flash-style attention: the [S,S] score matrix never
touches HBM).

Math per batch b (the 4 attention heads share one kernel, so compute 1 head):
  QT = Wq^T x_b^T, KT = Wk^T x_b^T, VT = Wv^T x_b^T          [100, 2048]
  scoresT[k,q] = sum_h KT[h,k] QT[h,q]                        per (k-tile, q-block)
  PT = exp(scoresT * 0.1)            (no max-subtraction: scores ~ N(0,1))
  AVT_aug = [V perm.+ ones col]^T @ PT -> [101, 512]; row 96 = denominator
  AVT = AVT_aug / denom              (rank-1 broadcast matmul + DVE multiply)
  a_outT = AD'^T AVT                 (AD' = head-folded attn_dense, row-permuted)
  h1T = relu(d1_w^T a_outT + d1_b)
  pooled = [max_q h1 | sum_q h1]     (mean's 1/2048 is folded into fc1_w rows)
  out = relu(fc2_w^T relu(fc1_w^T pooled + b1) + b2)

The per-q-block work is software-pipelined: the normalization + output
projection + d1 of block i are emitted after the scores/exp/AV accumulation of
block i+1, so the PE never idles on the reciprocal/broadcast chain.

All matmul operands are float32r (same fp32 bits; full-rate PE for free dims
>= 256 vs 4x slower plain fp32).  The denominator row sits at partition 96 (a
32-aligned offset, required for compute-engine PSUM access); V columns and
AD' rows are permuted to match (done on host).
"""

import numpy as np

import concourse.bacc as bacc
import concourse.mybir as mybir
import concourse.tile as tile
from concourse.bass_utils import run_bass_kernel_spmd

B, S, D, H = 16, 2048, 400, 100
DP = 512                 # d (=400) zero-padded to 4 partition tiles
NCORES = 8
BL = B // NCORES         # batch elems per core
SCALE = float(1.0 / np.float32(100.0) ** 0.5)
FP = mybir.dt.float32
FR = mybir.dt.float32r

NQB = S // 512           # 4 q-blocks of 512
NKT = S // 128           # 16 k-tiles of 128
NDT = DP // 128          # 4 d-tiles
OC = 96                  # ones-column / denominator position (32-aligned)


def build_kernel_body(tc, aps):
    nc = tc.nc
    Exp = mybir.ActivationFunctionType.Exp
    Relu = mybir.ActivationFunctionType.Relu
    X = mybir.AxisListType.X

    with (
        tc.tile_pool(name="params", bufs=1) as params,
        tc.tile_pool(name="xt", bufs=NDT) as xt_pool,
        tc.tile_pool(name="qkv", bufs=2) as qkv_pool,
        tc.tile_pool(name="vaug", bufs=2 * NKT) as vaug_pool,
        tc.tile_pool(name="pt", bufs=4) as pt_pool,
        tc.tile_pool(name="attn", bufs=2) as attn_pool,
        tc.tile_pool(name="ao", bufs=8) as ao_pool,
        tc.tile_pool(name="h1", bufs=3) as h1_pool,
        tc.tile_pool(name="stats", bufs=2) as stats_pool,
        tc.tile_pool(name="fc", bufs=1) as fc_pool,
        tc.tile_pool(name="sc", bufs=2, space="PSUM") as sc_pool,
        tc.tile_pool(name="ps", bufs=2, space="PSUM") as ps_pool,
        tc.tile_pool(name="acc", bufs=2, space="PSUM") as acc_pool,
    ):
        # ---- critical-path DMAs first: QKV weights, then batch-0 x^T ----
        wqkv_t = []
        for i in range(NDT):
            t = params.tile([128, 301], FR, name=f"wqkv{i}", tag=f"wqkv{i}")
            nc.sync.dma_start(t[:], aps["wqkv"][i * 128:(i + 1) * 128, :])
            wqkv_t.append(t)
        xtt = {}

        def load_xt(b):
            xtt[b] = []
            for i in range(NDT):
                t = xt_pool.tile([128, S], FR, name=f"xt{b}_{i}", tag="xt")
                nc.sync.dma_start(t[:], aps["xt"][b, i * 128:(i + 1) * 128, :])
                xtt[b].append(t)

        load_xt(0)

        # ---- remaining parameters (overlap with early compute) ----
        adp_sb = params.tile([H + 1, DP], FR, name="adp", tag="adp")
        nc.sync.dma_start(adp_sb[:], aps["adp"][:])
        d1w_t = []
        for i in range(NDT):
            t = params.tile([128, DP], FR, name=f"d1w{i}", tag=f"d1w{i}")
            nc.sync.dma_start(t[:], aps["d1w"][i * 128:(i + 1) * 128, :])
            d1w_t.append(t)
        d1b_sb = params.tile([128, 4], FP, name="d1b", tag="d1b")
        nc.sync.dma_start(d1b_sb[:], aps["d1b"][:])
        fc1w_t = []
        for i in range(8):
            t = params.tile([128, 512], FR, name=f"fc1w{i}", tag=f"fc1w{i}")
            nc.sync.dma_start(t[:], aps["fc1w"][i * 128:(i + 1) * 128, :])
            fc1w_t.append(t)
        fc1b_sb = params.tile([128, 4], FP, name="fc1b", tag="fc1b")
        nc.sync.dma_start(fc1b_sb[:], aps["fc1b"][:])
        fc2w_t = []
        for i in range(4):
            t = params.tile([128, 256], FR, name=f"fc2w{i}", tag=f"fc2w{i}")
            nc.sync.dma_start(t[:], aps["fc2w"][i * 128:(i + 1) * 128, :])
            fc2w_t.append(t)
        fc2b_sb = params.tile([128, 2], FP, name="fc2b", tag="fc2b")
        nc.sync.dma_start(fc2b_sb[:], aps["fc2b"][:])
        ident_sb = params.tile([128, 128], FP, name="ident", tag="ident")
        nc.sync.dma_start(ident_sb[:], aps["ident"][:])

        ones_sb = params.tile([1, H + 1], FP, name="ones", tag="ones")
        nc.vector.memset(ones_sb[:], 1.0)
        ones_fr = params.tile([1, H + 1], FR, name="onesfr", tag="onesfr")
        nc.vector.tensor_copy(ones_fr[:], ones_sb[:])
        onescol = params.tile([128, 1], FP, name="onescol", tag="onescol")
        nc.vector.memset(onescol[:], 1.0)

        # pooled feature vector, [128, BL] per p-tile: 0-3 max part, 4-7 sums
        pooled = [params.tile([128, BL], FP, name=f"pooled{p}",
                              tag=f"pooled{p}") for p in range(8)]

        state = {}

        def qkv_phase(b):
            qt = qkv_pool.tile([H, S], FR, name=f"qt{b}", tag="qt")
            kt = qkv_pool.tile([H, S], FR, name=f"kt{b}", tag="kt")
            # vt has 101 rows: V rows permuted with a spare row at OC (the
            # packed V weight block carries a zero column there)
            vt = qkv_pool.tile([H + 1, S], FP, name=f"vt{b}", tag="vt")
            proj = ((qt, 0, H), (kt, H, 2 * H), (vt, 2 * H, 3 * H + 1))
            for qb in range(NQB):
                sl = slice(qb * 512, (qb + 1) * 512)
                for pi, (dst, c0, c1) in enumerate(proj):
                    ps = ps_pool.tile([c1 - c0, 512], FP,
                                      name=f"qkvp{b}_{qb}_{pi}", tag="ps")
                    for di in range(NDT):
                        nc.tensor.matmul(
                            ps[:], wqkv_t[di][:, c0:c1],
                            xtt[b][di][:, sl],
                            start=(di == 0), stop=(di == NDT - 1))
                    nc.vector.tensor_copy(dst[:, sl], ps[:])

            # V^T -> V tiles; overwrite the spare column with ones
            vaug = []
            for s in range(NKT):
                pv = ps_pool.tile([128, H + 1], FP, name=f"vtp{b}_{s}",
                                  tag="ps")
                nc.tensor.transpose(pv[:], vt[:, s * 128:(s + 1) * 128],
                                    ident_sb[0:H + 1, 0:H + 1])
                va = vaug_pool.tile([128, H + 1], FR, name=f"vaug{b}_{s}",
                                    tag="vaug")
                nc.vector.tensor_copy(va[:], pv[:])
                nc.vector.tensor_copy(va[:, OC:OC + 1], onescol[:])
                vaug.append(va)

            bmax = [stats_pool.tile([128, NQB], FP, name=f"bmax{b}_{f}",
                                    tag=f"bmax{f}") for f in range(4)]
            bsum = [stats_pool.tile([128, NQB], FP, name=f"bsum{b}_{f}",
                                    tag=f"bsum{f}") for f in range(4)]
            state[b] = dict(qt=qt, kt=kt, vaug=vaug, bmax=bmax, bsum=bsum)

        def attention_block(b, qb):
            st = state[b]
            sl = slice(qb * 512, (qb + 1) * 512)
            avt = acc_pool.tile([H + 1, 512], FP, name=f"avt{b}_{qb}",
                                tag="avt")
            # k-tiles processed in pairs: two 512-wide score matmuls land in
            # one 2-bank PSUM region, one exp covers both (halves ACT
            # per-op overhead), then two AV accumulation matmuls.
            for kp in range(NKT // 2):
                k0, k1 = 2 * kp, 2 * kp + 1
                sc = sc_pool.tile([128, 1024], FP, name=f"sc{b}_{qb}_{kp}",
                                  tag="sc")
                nc.tensor.matmul(sc[:, 0:512],
                                 st["kt"][:, k0 * 128:(k0 + 1) * 128],
                                 st["qt"][:, sl], start=True, stop=True)
                nc.tensor.matmul(sc[:, 512:1024],
                                 st["kt"][:, k1 * 128:(k1 + 1) * 128],
                                 st["qt"][:, sl], start=True, stop=True)
                pt = pt_pool.tile([128, 1024], FR, name=f"pt{b}_{qb}_{kp}",
                                  tag="pt")
                nc.scalar.activation(pt[:], sc[:], Exp, scale=SCALE)
                nc.tensor.matmul(avt[:], st["vaug"][k0][:], pt[:, 0:512],
                                 start=(kp == 0), stop=False)
                nc.tensor.matmul(avt[:], st["vaug"][k1][:], pt[:, 512:1024],
                                 start=False, stop=(kp == NKT // 2 - 1))
            return avt

        def finish_block(b, qb, avt):
            st = state[b]
            # softmax normalization: 1/denom as exp(-ln(denom)) on ACT
            # (Ln and Exp share one activation table set; DVE's exact
            # reciprocal is 8 cycles/element and far slower)
            lnd = attn_pool.tile([1, 512], FP, name=f"lnd{b}_{qb}", tag="lnd")
            nc.scalar.activation(lnd[:], avt[OC:OC + 1, :],
                                 mybir.ActivationFunctionType.Ln)
            recip_r = attn_pool.tile([1, 512], FR, name=f"recr{b}_{qb}",
                                     tag="recr")
            nc.scalar.activation(recip_r[:], lnd[:], Exp, scale=-1.0)
            bcp = ps_pool.tile([H + 1, 512], FP, name=f"bcp{b}_{qb}", tag="ps")
            nc.tensor.matmul(bcp[:], ones_fr[:], recip_r[:],
                             start=True, stop=True)
            bc = attn_pool.tile([H + 1, 512], FP, name=f"bc{b}_{qb}", tag="bc")
            nc.vector.tensor_copy(bc[:], bcp[:])
            avtn = attn_pool.tile([H + 1, 512], FR, name=f"avtn{b}_{qb}",
                                  tag="avtn")
            nc.vector.tensor_tensor(avtn[:], avt[:], bc[:],
                                    op=mybir.AluOpType.mult)

            # attention output projection (head-folded, row-permuted AD')
            ao = []
            for e in range(NDT):
                pa = ps_pool.tile([128, 512], FP, name=f"ao{b}_{qb}_{e}",
                                  tag="ps")
                nc.tensor.matmul(pa[:], adp_sb[:, e * 128:(e + 1) * 128],
                                 avtn[:], start=True, stop=True)
                at = ao_pool.tile([128, 512], FR, name=f"aot{b}_{qb}_{e}",
                                  tag="ao")
                nc.vector.tensor_copy(at[:], pa[:])
                ao.append(at)

            # d1 + bias + relu; row-sums accumulated for mean pooling
            for f in range(4):
                ph = ps_pool.tile([128, 512], FP, name=f"h1p{b}_{qb}_{f}",
                                  tag="ps")
                for e in range(NDT):
                    nc.tensor.matmul(ph[:],
                                     d1w_t[e][:, f * 128:(f + 1) * 128],
                                     ao[e][:], start=(e == 0),
                                     stop=(e == NDT - 1))
                h1 = h1_pool.tile([128, 512], FP, name=f"h1{b}_{qb}_{f}",
                                  tag="h1")
                nc.scalar.activation(
                    h1[:], ph[:], Relu, bias=d1b_sb[:, f:f + 1],
                    accum_out=st["bsum"][f][:, qb:qb + 1])
                nc.vector.reduce_max(st["bmax"][f][:, qb:qb + 1], h1[:],
                                     axis=X)

            if qb == NQB - 1:
                for f in range(4):
                    nc.vector.reduce_max(pooled[f][:, b:b + 1],
                                         st["bmax"][f][:], axis=X)
                    nc.vector.reduce_sum(pooled[4 + f][:, b:b + 1],
                                         st["bsum"][f][:], axis=X)

        # ---- pipelined emission over all (batch, q-block) pairs ----
        prev = None
        for b in range(BL):
            if b > 0:
                load_xt(b)
            qkv_phase(b)
            for qb in range(NQB):
                avt = attention_block(b, qb)
                if prev is not None:
                    finish_block(*prev)
                prev = (b, qb, avt)
        finish_block(*prev)

        # rounded copies of pooled for the f32r fc matmuls
        pooled_r = []
        for p in range(8):
            t = params.tile([128, BL], FR, name=f"pooledr{p}",
                            tag=f"pooledr{p}")
            nc.vector.tensor_copy(t[:], pooled[p][:])
            pooled_r.append(t)

        # ---- fc head (both batches together; N = BL columns) ----
        pf1 = ps_pool.tile([BL, 512], FP, name="fc1ps", tag="ps")
        for p in range(8):
            nc.tensor.matmul(pf1[:], pooled_r[p][:], fc1w_t[p][:],
                             start=(p == 0), stop=(p == 7))
        o1 = fc_pool.tile([BL, 512], FP, name="o1", tag="o1")
        nc.vector.tensor_copy(o1[:], pf1[:])
        h1fc = []
        for g in range(4):
            pt_ = ps_pool.tile([128, BL], FP, name=f"fct{g}", tag="ps")
            nc.tensor.transpose(pt_[:], o1[:, g * 128:(g + 1) * 128],
                                ident_sb[0:BL, 0:BL])
            hg = fc_pool.tile([128, BL], FR, name=f"h1fc{g}", tag=f"h1fc{g}")
            nc.scalar.activation(hg[:], pt_[:], Relu, bias=fc1b_sb[:, g:g + 1])
            h1fc.append(hg)

        pf2 = ps_pool.tile([BL, 256], FP, name="fc2ps", tag="ps")
        for g in range(4):
            nc.tensor.matmul(pf2[:], h1fc[g][:], fc2w_t[g][:],
                             start=(g == 0), stop=(g == 3))
        o2 = fc_pool.tile([BL, 256], FP, name="o2", tag="o2")
        nc.vector.tensor_copy(o2[:], pf2[:])
        for o in range(2):
            pt2 = ps_pool.tile([128, BL], FP, name=f"fct2{o}", tag="ps")
            nc.tensor.transpose(pt2[:], o2[:, o * 128:(o + 1) * 128],
                                ident_sb[0:BL, 0:BL])
            ot = fc_pool.tile([128, BL], FP, name=f"ot{o}", tag=f"ot{o}")
            nc.scalar.activation(ot[:], pt2[:], Relu, bias=fc2b_sb[:, o:o + 1])
            nc.sync.dma_start(aps["out"][o * 128:(o + 1) * 128, :], ot[:])


def build_nc():
    # Bacc (not raw Bass): finalize() runs move_matmul_waits_to_ldweights +
    # generate_event_semaphores, legalizing multi-wait instructions for the
    # 1-wait-slot self-loading fp32/f32r matmul struct.
    nc = bacc.Bacc("TRN2", target_bir_lowering=False, debug=False,
                   num_devices=NCORES)
    aps = {
        "xt": nc.dram_tensor("xt", [BL, DP, S], FR, kind="ExternalInput").ap(),
        "wqkv": nc.dram_tensor("wqkv", [DP, 3 * H + 1], FR,
                               kind="ExternalInput").ap(),
        "adp": nc.dram_tensor("adp", [H + 1, DP], FR,
                              kind="ExternalInput").ap(),
        "d1w": nc.dram_tensor("d1w", [DP, DP], FR, kind="ExternalInput").ap(),
        "d1b": nc.dram_tensor("d1b", [128, 4], FP, kind="ExternalInput").ap(),
        "fc1w": nc.dram_tensor("fc1w", [1024, 512], FR,
                               kind="ExternalInput").ap(),
        "fc1b": nc.dram_tensor("fc1b", [128, 4], FP,
                               kind="ExternalInput").ap(),
        "fc2w": nc.dram_tensor("fc2w", [512, 256], FR,
                               kind="ExternalInput").ap(),
        "fc2b": nc.dram_tensor("fc2b", [128, 2], FP,
                               kind="ExternalInput").ap(),
        "ident": nc.dram_tensor("ident", [128, 128], FP,
                                kind="ExternalInput").ap(),
        "out": nc.dram_tensor("out", [256, BL], FP, kind="ExternalOutput").ap(),
    }
    with tile.TileContext(nc) as tc:
        build_kernel_body(tc, aps)
    nc.finalize()
    return nc


_NC_CACHE = None


def get_nc():
    global _NC_CACHE
    if _NC_CACHE is None:
        _NC_CACHE = build_nc()
    return _NC_CACHE


def prep_inputs(x, attn_kernel, attn_dense, d1_w, d1_b, fc1_w, fc1_b,
                fc2_w, fc2_b):
    """Host-side layout prep. Returns per-core input maps."""
    x = np.ascontiguousarray(np.asarray(x, np.float32))
    attn_kernel = np.asarray(attn_kernel, np.float32)
    attn_dense = np.asarray(attn_dense, np.float32)
    d1_w = np.asarray(d1_w, np.float32)
    d1_b = np.asarray(d1_b, np.float32)
    fc1_w = np.asarray(fc1_w, np.float32)
    fc1_b = np.asarray(fc1_b, np.float32)
    fc2_w = np.asarray(fc2_w, np.float32)
    fc2_b = np.asarray(fc2_b, np.float32)

    xt = np.zeros((B, DP, S), np.float32)
    xt[:, :D, :] = x.transpose(0, 2, 1)

    wqkv = np.zeros((DP, 3 * H + 1), np.float32)
    wqkv[:D, 0:H] = attn_kernel[0]
    wqkv[:D, H:2 * H] = attn_kernel[1]
    # V block: 101 columns with a zero column at OC (ones slot), so V^T comes
    # out of the projection already in the permuted 101-row layout
    wqkv[:D, 2 * H:2 * H + OC] = attn_kernel[2][:, 0:OC]
    wqkv[:D, 2 * H + OC + 1:3 * H + 1] = attn_kernel[2][:, OC:H]

    ad_folded = (attn_dense[0:H] + attn_dense[H:2 * H]
                 + attn_dense[2 * H:3 * H] + attn_dense[3 * H:4 * H])
    # rows permuted to match the V_aug column order (ones column at OC)
    adp = np.zeros((H + 1, DP), np.float32)
    adp[0:OC, :D] = ad_folded[0:OC]
    adp[OC + 1:H + 1, :D] = ad_folded[OC:H]

    d1w = np.zeros((DP, DP), np.float32)
    d1w[:D, :D] = d1_w
    d1b = np.zeros((DP,), np.float32)
    d1b[:D] = d1_b
    d1b_t = np.ascontiguousarray(d1b.reshape(4, 128).T)

    fc1w = np.zeros((2 * DP, 512), np.float32)
    fc1w[0:D] = fc1_w[0:D]
    fc1w[DP:DP + D] = fc1_w[D:2 * D] * np.float32(1.0 / S)
    fc1b_t = np.ascontiguousarray(fc1_b.reshape(4, 128).T)
    fc2b_t = np.ascontiguousarray(fc2_b.reshape(2, 128).T)

    ident = np.eye(128, dtype=np.float32)

    shared = dict(wqkv=wqkv, adp=adp, d1w=d1w, d1b=d1b_t, fc1w=fc1w,
                  fc1b=fc1b_t, fc2w=np.ascontiguousarray(fc2_w),
                  fc2b=fc2b_t, ident=ident)
    in_maps = []
    for c in range(NCORES):
        m = dict(shared)
        m["xt"] = np.ascontiguousarray(xt[c * BL:(c + 1) * BL])
        in_maps.append(m)
    return in_maps


def kernel(**inputs):
    in_maps = prep_inputs(**inputs)
    nc = get_nc()
    res = run_bass_kernel_spmd(nc, in_maps, core_ids=list(range(NCORES)))
    outs = [res.results[c]["out"] for c in range(NCORES)]
    return np.ascontiguousarray(
        np.concatenate([o.T for o in outs], axis=0).astype(np.float32))


# revision 23
# speedup vs baseline: 1.1289x; 1.1289x over previous
"""Trainium2 Bass kernel for nn_ExpertModule (dense transformer block).

Strategy: data-parallel over batch across 8 NeuronCores (16 batches -> 2 per
core), small parameter set replicated to every core.  All heavy compute stays
on-chip per batch element (flash-style attention: the [S,S] score matrix never
touches HBM).

Math per batch b (the 4 attention heads share one kernel, so compute 1 head):
  QT = Wq^T x_b^T, KT = Wk^T x_b^T, VT = Wv^T x_b^T          [100, 2048]
  scoresT[k,q] = sum_h KT[h,k] QT[h,q]                        per (k-tile, q-block)
  PT = exp(scoresT * 0.1)            (no max-subtraction: scores ~ N(0,1))
  AVT_aug = [V perm.+ ones col]^T @ PT -> [101, 512]; row 96 = denominator
  AVT = AVT_aug / denom              (rank-1 broadcast matmul + DVE multiply)
  a_outT = AD'^T AVT                 (AD' = head-folded attn_dense, row-permuted)
  h1T = relu(d1_w^T a_outT + d1_b)
  pooled = [max_q h1 | sum_q h1]     (mean's 1/2048 is folded into fc1_w rows)
  out = relu(fc2_w^T relu(fc1_w^T pooled + b1) + b2)

The per-q-block work is software-pipelined: the normalization + output
projection + d1 of block i are emitted after the scores/exp/AV accumulation of
block i+1, so the PE never idles on the reciprocal/broadcast chain.

All matmul operands are float32r (same fp32 bits; full-rate PE for free dims
>= 256 vs 4x slower plain fp32).  The denominator row sits at partition 96 (a
32-aligned offset, required for compute-engine PSUM access); V columns and
AD' rows are permuted to match (done on host).
"""

import numpy as np

import concourse.bacc as bacc
import concourse.mybir as mybir
import concourse.tile as tile
from concourse.bass_utils import run_bass_kernel_spmd

B, S, D, H = 16, 2048, 400, 100
DP = 512                 # d (=400) zero-padded to 4 partition tiles
NCORES = 8
BL = B // NCORES         # batch elems per core
SCALE = float(1.0 / np.float32(100.0) ** 0.5)
FP = mybir.dt.float32
FR = mybir.dt.float32r

NQB = S // 512           # 4 q-blocks of 512
NKT = S // 128           # 16 k-tiles of 128
NDT = DP // 128          # 4 d-tiles
OC = 96                  # ones-column / denominator position (32-aligned)


def build_kernel_body(tc, aps):
    nc = tc.nc
    Exp = mybir.ActivationFunctionType.Exp
    Relu = mybir.ActivationFunctionType.Relu
    X = mybir.AxisListType.X

    with (
        tc.tile_pool(name="params", bufs=1) as params,
        tc.tile_pool(name="xt", bufs=NDT) as xt_pool,
        tc.tile_pool(name="qkv", bufs=2) as qkv_pool,
        tc.tile_pool(name="vaug", bufs=2 * NKT) as vaug_pool,
        tc.tile_pool(name="pt", bufs=4) as pt_pool,
        tc.tile_pool(name="attn", bufs=2) as attn_pool,
        tc.tile_pool(name="ao", bufs=8) as ao_pool,
        tc.tile_pool(name="h1", bufs=3) as h1_pool,
        tc.tile_pool(name="stats", bufs=2) as stats_pool,
        tc.tile_pool(name="fc", bufs=1) as fc_pool,
        tc.tile_pool(name="sc", bufs=2, space="PSUM") as sc_pool,
        tc.tile_pool(name="ps", bufs=2, space="PSUM") as ps_pool,
        tc.tile_pool(name="acc", bufs=2, space="PSUM") as acc_pool,
    ):
        # ---- critical-path DMAs first: QKV weights, then batch-0 x^T ----
        wqkv_t = []
        for i in range(NDT):
            t = params.tile([128, 301], FR, name=f"wqkv{i}", tag=f"wqkv{i}")
            nc.sync.dma_start(t[:], aps["wqkv"][i * 128:(i + 1) * 128, :])
            wqkv_t.append(t)
        xtt = {}

        def load_xt(b):
            xtt[b] = []
            for i in range(NDT):
                t = xt_pool.tile([128, S], FR, name=f"xt{b}_{i}", tag="xt")
                nc.sync.dma_start(t[:], aps["xt"][b, i * 128:(i + 1) * 128, :])
                xtt[b].append(t)

        load_xt(0)

        # ---- remaining parameters (overlap with early compute) ----
        adp_sb = params.tile([H + 1, DP], FR, name="adp", tag="adp")
        nc.sync.dma_start(adp_sb[:], aps["adp"][:])
        d1w_t = []
        for i in range(NDT):
            t = params.tile([128, DP], FR, name=f"d1w{i}", tag=f"d1w{i}")
            nc.sync.dma_start(t[:], aps["d1w"][i * 128:(i + 1) * 128, :])
            d1w_t.append(t)
        d1b_sb = params.tile([128, 4], FP, name="d1b", tag="d1b")
        nc.sync.dma_start(d1b_sb[:], aps["d1b"][:])
        fc1w_t = []
        for i in range(8):
            t = params.tile([128, 512], FR, name=f"fc1w{i}", tag=f"fc1w{i}")
            nc.sync.dma_start(t[:], aps["fc1w"][i * 128:(i + 1) * 128, :])
            fc1w_t.append(t)
        fc1b_sb = params.tile([128, 4], FP, name="fc1b", tag="fc1b")
        nc.sync.dma_start(fc1b_sb[:], aps["fc1b"][:])
        fc2w_t = []
        for i in range(4):
            t = params.tile([128, 256], FR, name=f"fc2w{i}", tag=f"fc2w{i}")
            nc.sync.dma_start(t[:], aps["fc2w"][i * 128:(i + 1) * 128, :])
            fc2w_t.append(t)
        fc2b_sb = params.tile([128, 2], FP, name="fc2b", tag="fc2b")
        nc.sync.dma_start(fc2b_sb[:], aps["fc2b"][:])
        ident_sb = params.tile([128, 128], FP, name="ident", tag="ident")
        nc.sync.dma_start(ident_sb[:], aps["ident"][:])

        ones_sb = params.tile([1, H + 1], FP, name="ones", tag="ones")
        nc.vector.memset(ones_sb[:], 1.0)
        ones_fr = params.tile([1, H + 1], FR, name="onesfr", tag="onesfr")
        nc.vector.tensor_copy(ones_fr[:], ones_sb[:])
        onescol = params.tile([128, 1], FP, name="onescol", tag="onescol")
        nc.vector.memset(onescol[:], 1.0)
        zeros_sb = params.tile([128, 512], FP, name="zeros", tag="zeros")
        nc.vector.memset(zeros_sb[:], 0.0)

        # pooled feature vector, [128, BL] per p-tile: 0-3 max part, 4-7 sums
        pooled = [params.tile([128, BL], FP, name=f"pooled{p}",
                              tag=f"pooled{p}") for p in range(8)]

        state = {}

        def qkv_phase(b):
            qt = qkv_pool.tile([H, S], FR, name=f"qt{b}", tag="qt")
            kt = qkv_pool.tile([H, S], FR, name=f"kt{b}", tag="kt")
            # vt has 101 rows: V rows permuted with a spare row at OC (the
            # packed V weight block carries a zero column there)
            vt = qkv_pool.tile([H + 1, S], FP, name=f"vt{b}", tag="vt")
            proj = ((qt, 0, H), (kt, H, 2 * H), (vt, 2 * H, 3 * H + 1))
            for qb in range(NQB):
                sl = slice(qb * 512, (qb + 1) * 512)
                for pi, (dst, c0, c1) in enumerate(proj):
                    ps = ps_pool.tile([c1 - c0, 512], FP,
                                      name=f"qkvp{b}_{qb}_{pi}", tag="ps")
                    for di in range(NDT):
                        nc.tensor.matmul(
                            ps[:], wqkv_t[di][:, c0:c1],
                            xtt[b][di][:, sl],
                            start=(di == 0), stop=(di == NDT - 1))
                    nc.vector.tensor_copy(dst[:, sl], ps[:])

            # V^T -> V tiles; overwrite the spare column with ones
            vaug = []
            for s in range(NKT):
                pv = ps_pool.tile([128, H + 1], FP, name=f"vtp{b}_{s}",
                                  tag="ps")
                nc.tensor.transpose(pv[:], vt[:, s * 128:(s + 1) * 128],
                                    ident_sb[0:H + 1, 0:H + 1])
                va = vaug_pool.tile([128, H + 1], FR, name=f"vaug{b}_{s}",
                                    tag="vaug")
                nc.vector.tensor_copy(va[:], pv[:])
                nc.vector.tensor_copy(va[:, OC:OC + 1], onescol[:])
                vaug.append(va)

            bmax = [stats_pool.tile([128, NQB], FP, name=f"bmax{b}_{f}",
                                    tag=f"bmax{f}") for f in range(4)]
            bsum = [stats_pool.tile([128, NQB], FP, name=f"bsum{b}_{f}",
                                    tag=f"bsum{f}") for f in range(4)]
            state[b] = dict(qt=qt, kt=kt, vaug=vaug, bmax=bmax, bsum=bsum)

        def attention_block(b, qb):
            st = state[b]
            sl = slice(qb * 512, (qb + 1) * 512)
            avt = acc_pool.tile([H + 1, 512], FP, name=f"avt{b}_{qb}",
                                tag="avt")
            # k-tiles processed in pairs: two 512-wide score matmuls land in
            # one 2-bank PSUM region, one exp covers both (halves ACT
            # per-op overhead), then two AV accumulation matmuls.
            for kp in range(NKT // 2):
                k0, k1 = 2 * kp, 2 * kp + 1
                sc = sc_pool.tile([128, 1024], FP, name=f"sc{b}_{qb}_{kp}",
                                  tag="sc")
                nc.tensor.matmul(sc[:, 0:512],
                                 st["kt"][:, k0 * 128:(k0 + 1) * 128],
                                 st["qt"][:, sl], start=True, stop=True)
                nc.tensor.matmul(sc[:, 512:1024],
                                 st["kt"][:, k1 * 128:(k1 + 1) * 128],
                                 st["qt"][:, sl], start=True, stop=True)
                pt = pt_pool.tile([128, 1024], FR, name=f"pt{b}_{qb}_{kp}",
                                  tag="pt")
                nc.scalar.activation(pt[:], sc[:], Exp, scale=SCALE)
                nc.tensor.matmul(avt[:], st["vaug"][k0][:], pt[:, 0:512],
                                 start=(kp == 0), stop=False)
                nc.tensor.matmul(avt[:], st["vaug"][k1][:], pt[:, 512:1024],
                                 start=False, stop=(kp == NKT // 2 - 1))
            return avt

        def finish_block(b, qb, avt):
            st = state[b]
            # softmax normalization: 1/denom as exp(-ln(denom)) on ACT
            # (Ln and Exp share one activation table set; DVE's exact
            # reciprocal is 8 cycles/element and far slower)
            lnd = attn_pool.tile([1, 512], FP, name=f"lnd{b}_{qb}", tag="lnd")
            nc.scalar.activation(lnd[:], avt[OC:OC + 1, :],
                                 mybir.ActivationFunctionType.Ln)
            recip_r = attn_pool.tile([1, 512], FR, name=f"recr{b}_{qb}",
                                     tag="recr")
            nc.scalar.activation(recip_r[:], lnd[:], Exp, scale=-1.0)
            bcp = ps_pool.tile([H + 1, 512], FP, name=f"bcp{b}_{qb}", tag="ps")
            nc.tensor.matmul(bcp[:], ones_fr[:], recip_r[:],
                             start=True, stop=True)
            bc = attn_pool.tile([H + 1, 512], FP, name=f"bc{b}_{qb}", tag="bc")
            nc.vector.tensor_copy(bc[:], bcp[:])
            avtn = attn_pool.tile([H + 1, 512], FR, name=f"avtn{b}_{qb}",
                                  tag="avtn")
            nc.vector.tensor_tensor(avtn[:], avt[:], bc[:],
                                    op=mybir.AluOpType.mult)

            # attention output projection (head-folded, row-permuted AD')
            ao = []
            for e in range(NDT):
                pa = ps_pool.tile([128, 512], FP, name=f"ao{b}_{qb}_{e}",
                                  tag="ps")
                nc.tensor.matmul(pa[:], adp_sb[:, e * 128:(e + 1) * 128],
                                 avtn[:], start=True, stop=True)
                at = ao_pool.tile([128, 512], FR, name=f"aot{b}_{qb}_{e}",
                                  tag="ao")
                nc.vector.tensor_copy(at[:], pa[:])
                ao.append(at)

            # d1 + bias + relu; row-sums accumulated for mean pooling
            for f in range(4):
                ph = ps_pool.tile([128, 512], FP, name=f"h1p{b}_{qb}_{f}",
                                  tag="ps")
                for e in range(NDT):
                    nc.tensor.matmul(ph[:],
                                     d1w_t[e][:, f * 128:(f + 1) * 128],
                                     ao[e][:], start=(e == 0),
                                     stop=(e == NDT - 1))
                h1 = h1_pool.tile([128, 512], FP, name=f"h1{b}_{qb}_{f}",
                                  tag="h1")
                # relu(ph + bias) with fused row-sum, on DVE (ACT is busy
                # with exp; this keeps d1 PSUM evacuation off its queue)
                nc.vector.scalar_tensor_tensor(
                    h1[:], ph[:], d1b_sb[:, f:f + 1], zeros_sb[:],
                    op0=mybir.AluOpType.add, op1=mybir.AluOpType.max,
                    accum_out=st["bsum"][f][:, qb:qb + 1])
                nc.vector.reduce_max(st["bmax"][f][:, qb:qb + 1], h1[:],
                                     axis=X)

            if qb == NQB - 1:
                for f in range(4):
                    nc.vector.reduce_max(pooled[f][:, b:b + 1],
                                         st["bmax"][f][:], axis=X)
                    nc.vector.reduce_sum(pooled[4 + f][:, b:b + 1],
                                         st["bsum"][f][:], axis=X)

        # ---- pipelined emission over all (batch, q-block) pairs ----
        prev = None
        for b in range(BL):
            if b > 0:
                load_xt(b)
            qkv_phase(b)
            for qb in range(NQB):
                avt = attention_block(b, qb)
                if prev is not None:
                    finish_block(*prev)
                prev = (b, qb, avt)
        finish_block(*prev)

        # rounded copies of pooled for the f32r fc matmuls
        pooled_r = []
        for p in range(8):
            t = params.tile([128, BL], FR, name=f"pooledr{p}",
                            tag=f"pooledr{p}")
            nc.vector.tensor_copy(t[:], pooled[p][:])
            pooled_r.append(t)

        # ---- fc head (both batches together; N = BL columns) ----
        pf1 = ps_pool.tile([BL, 512], FP, name="fc1ps", tag="ps")
        for p in range(8):
            nc.tensor.matmul(pf1[:], pooled_r[p][:], fc1w_t[p][:],
                             start=(p == 0), stop=(p == 7))
        o1 = fc_pool.tile([BL, 512], FP, name="o1", tag="o1")
        nc.vector.tensor_copy(o1[:], pf1[:])
        h1fc = []
        for g in range(4):
            pt_ = ps_pool.tile([128, BL], FP, name=f"fct{g}", tag="ps")
            nc.tensor.transpose(pt_[:], o1[:, g * 128:(g + 1) * 128],
                                ident_sb[0:BL, 0:BL])
            hg = fc_pool.tile([128, BL], FR, name=f"h1fc{g}", tag=f"h1fc{g}")
            nc.scalar.activation(hg[:], pt_[:], Relu, bias=fc1b_sb[:, g:g + 1])
            h1fc.append(hg)

        pf2 = ps_pool.tile([BL, 256], FP, name="fc2ps", tag="ps")
        for g in range(4):
            nc.tensor.matmul(pf2[:], h1fc[g][:], fc2w_t[g][:],
                             start=(g == 0), stop=(g == 3))
        o2 = fc_pool.tile([BL, 256], FP, name="o2", tag="o2")
        nc.vector.tensor_copy(o2[:], pf2[:])
        for o in range(2):
            pt2 = ps_pool.tile([128, BL], FP, name=f"fct2{o}", tag="ps")
            nc.tensor.transpose(pt2[:], o2[:, o * 128:(o + 1) * 128],
                                ident_sb[0:BL, 0:BL])
            ot = fc_pool.tile([128, BL], FP, name=f"ot{o}", tag=f"ot{o}")
            nc.scalar.activation(ot[:], pt2[:], Relu, bias=fc2b_sb[:, o:o + 1])
            nc.sync.dma_start(aps["out"][o * 128:(o + 1) * 128, :], ot[:])


def _pin_activation_table(arch="gen3"):
    """All activations used here (exp, ln, relu) live in one table set
    (natural_log_exp_and_others).  The table-load inserter picks the first
    set containing each function, which ping-pongs exp<->ln sets and costs
    a ~1.3us ACT_TABLE_LOAD per switch.  Empty every other set in the cached
    table dict (indices preserved) so exactly one load is ever emitted."""
    import concourse.hw_specs as hw_specs
    tables = hw_specs.get_activation_tables(arch)
    for name, funcs in tables.items():
        if name != "natural_log_exp_and_others":
            funcs.clear()


def build_nc():
    # Bacc (not raw Bass): finalize() runs move_matmul_waits_to_ldweights +
    # generate_event_semaphores, legalizing multi-wait instructions for the
    # 1-wait-slot self-loading fp32/f32r matmul struct.
    nc = bacc.Bacc("TRN2", target_bir_lowering=False, debug=False,
                   num_devices=NCORES)
    _pin_activation_table(nc.m.arch)
    aps = {
        "xt": nc.dram_tensor("xt", [BL, DP, S], FR, kind="ExternalInput").ap(),
        "wqkv": nc.dram_tensor("wqkv", [DP, 3 * H + 1], FR,
                               kind="ExternalInput").ap(),
        "adp": nc.dram_tensor("adp", [H + 1, DP], FR,
                              kind="ExternalInput").ap(),
        "d1w": nc.dram_tensor("d1w", [DP, DP], FR, kind="ExternalInput").ap(),
        "d1b": nc.dram_tensor("d1b", [128, 4], FP, kind="ExternalInput").ap(),
        "fc1w": nc.dram_tensor("fc1w", [1024, 512], FR,
                               kind="ExternalInput").ap(),
        "fc1b": nc.dram_tensor("fc1b", [128, 4], FP,
                               kind="ExternalInput").ap(),
        "fc2w": nc.dram_tensor("fc2w", [512, 256], FR,
                               kind="ExternalInput").ap(),
        "fc2b": nc.dram_tensor("fc2b", [128, 2], FP,
                               kind="ExternalInput").ap(),
        "ident": nc.dram_tensor("ident", [128, 128], FP,
                                kind="ExternalInput").ap(),
        "out": nc.dram_tensor("out", [256, BL], FP, kind="ExternalOutput").ap(),
    }
    with tile.TileContext(nc) as tc:
        build_kernel_body(tc, aps)
    nc.finalize()
    return nc


_NC_CACHE = None


def get_nc():
    global _NC_CACHE
    if _NC_CACHE is None:
        _NC_CACHE = build_nc()
    return _NC_CACHE


def prep_inputs(x, attn_kernel, attn_dense, d1_w, d1_b, fc1_w, fc1_b,
                fc2_w, fc2_b):
    """Host-side layout prep. Returns per-core input maps."""
    x = np.ascontiguousarray(np.asarray(x, np.float32))
    attn_kernel = np.asarray(attn_kernel, np.float32)
    attn_dense = np.asarray(attn_dense, np.float32)
    d1_w = np.asarray(d1_w, np.float32)
    d1_b = np.asarray(d1_b, np.float32)
    fc1_w = np.asarray(fc1_w, np.float32)
    fc1_b = np.asarray(fc1_b, np.float32)
    fc2_w = np.asarray(fc2_w, np.float32)
    fc2_b = np.asarray(fc2_b, np.float32)

    xt = np.zeros((B, DP, S), np.float32)
    xt[:, :D, :] = x.transpose(0, 2, 1)

    wqkv = np.zeros((DP, 3 * H + 1), np.float32)
    wqkv[:D, 0:H] = attn_kernel[0]
    wqkv[:D, H:2 * H] = attn_kernel[1]
    # V block: 101 columns with a zero column at OC (ones slot), so V^T comes
    # out of the projection already in the permuted 101-row layout
    wqkv[:D, 2 * H:2 * H + OC] = attn_kernel[2][:, 0:OC]
    wqkv[:D, 2 * H + OC + 1:3 * H + 1] = attn_kernel[2][:, OC:H]

    ad_folded = (attn_dense[0:H] + attn_dense[H:2 * H]
                 + attn_dense[2 * H:3 * H] + attn_dense[3 * H:4 * H])
    # rows permuted to match the V_aug column order (ones column at OC)
    adp = np.zeros((H + 1, DP), np.float32)
    adp[0:OC, :D] = ad_folded[0:OC]
    adp[OC + 1:H + 1, :D] = ad_folded[OC:H]

    d1w = np.zeros((DP, DP), np.float32)
    d1w[:D, :D] = d1_w
    d1b = np.zeros((DP,), np.float32)
    d1b[:D] = d1_b
    d1b_t = np.ascontiguousarray(d1b.reshape(4, 128).T)

    fc1w = np.zeros((2 * DP, 512), np.float32)
    fc1w[0:D] = fc1_w[0:D]
    fc1w[DP:DP + D] = fc1_w[D:2 * D] * np.float32(1.0 / S)
    fc1b_t = np.ascontiguousarray(fc1_b.reshape(4, 128).T)
    fc2b_t = np.ascontiguousarray(fc2_b.reshape(2, 128).T)

    ident = np.eye(128, dtype=np.float32)

    shared = dict(wqkv=wqkv, adp=adp, d1w=d1w, d1b=d1b_t, fc1w=fc1w,
                  fc1b=fc1b_t, fc2w=np.ascontiguousarray(fc2_w),
                  fc2b=fc2b_t, ident=ident)
    in_maps = []
    for c in range(NCORES):
        m = dict(shared)
        m["xt"] = np.ascontiguousarray(xt[c * BL:(c + 1) * BL])
        in_maps.append(m)
    return in_maps


def kernel(**inputs):
    in_maps = prep_inputs(**inputs)
    nc = get_nc()
    res = run_bass_kernel_spmd(nc, in_maps, core_ids=list(range(NCORES)))
    outs = [res.results[c]["out"] for c in range(NCORES)]
    return np.ascontiguousarray(
        np.concatenate([o.T for o in outs], axis=0).astype(np.float32))


# revision 24
# speedup vs baseline: 1.1423x; 1.0118x over previous
"""Trainium2 Bass kernel for nn_ExpertModule (dense transformer block).

Strategy: data-parallel over batch across 8 NeuronCores (16 batches -> 2 per
core), small parameter set replicated to every core.  All heavy compute stays
on-chip per batch element (flash-style attention: the [S,S] score matrix never
touches HBM).

Math per batch b (the 4 attention heads share one kernel, so compute 1 head):
  QT = Wq^T x_b^T, KT = Wk^T x_b^T, VT = Wv^T x_b^T          [100, 2048]
  scoresT[k,q] = sum_h KT[h,k] QT[h,q]                        per (k-tile, q-block)
  PT = exp(scoresT * 0.1)            (no max-subtraction: scores ~ N(0,1))
  AVT_aug = [V perm.+ ones col]^T @ PT -> [101, 512]; row 96 = denominator
  AVT = AVT_aug / denom              (rank-1 broadcast matmul + DVE multiply)
  a_outT = AD'^T AVT                 (AD' = head-folded attn_dense, row-permuted)
  h1T = relu(d1_w^T a_outT + d1_b)
  pooled = [max_q h1 | sum_q h1]     (mean's 1/2048 is folded into fc1_w rows)
  out = relu(fc2_w^T relu(fc1_w^T pooled + b1) + b2)

The per-q-block work is software-pipelined: the normalization + output
projection + d1 of block i are emitted after the scores/exp/AV accumulation of
block i+1, so the PE never idles on the reciprocal/broadcast chain.

All matmul operands are float32r (same fp32 bits; full-rate PE for free dims
>= 256 vs 4x slower plain fp32).  The denominator row sits at partition 96 (a
32-aligned offset, required for compute-engine PSUM access); V columns and
AD' rows are permuted to match (done on host).
"""

import numpy as np

import concourse.bacc as bacc
import concourse.mybir as mybir
import concourse.tile as tile
from concourse.bass_utils import run_bass_kernel_spmd

B, S, D, H = 16, 2048, 400, 100
DP = 512                 # d (=400) zero-padded to 4 partition tiles
NCORES = 8
BL = B // NCORES         # batch elems per core
SCALE = float(1.0 / np.float32(100.0) ** 0.5)
FP = mybir.dt.float32
FR = mybir.dt.float32r

NQB = S // 512           # 4 q-blocks of 512
NKT = S // 128           # 16 k-tiles of 128
NDT = DP // 128          # 4 d-tiles
OC = 96                  # ones-column / denominator position (32-aligned)


def build_kernel_body(tc, aps):
    nc = tc.nc
    Exp = mybir.ActivationFunctionType.Exp
    Relu = mybir.ActivationFunctionType.Relu
    X = mybir.AxisListType.X

    with (
        tc.tile_pool(name="params", bufs=1) as params,
        tc.tile_pool(name="xt", bufs=NDT) as xt_pool,
        tc.tile_pool(name="qkv", bufs=2) as qkv_pool,
        tc.tile_pool(name="vaug", bufs=2 * NKT) as vaug_pool,
        tc.tile_pool(name="pt", bufs=4) as pt_pool,
        tc.tile_pool(name="attn", bufs=2) as attn_pool,
        tc.tile_pool(name="ao", bufs=8) as ao_pool,
        tc.tile_pool(name="h1", bufs=3) as h1_pool,
        tc.tile_pool(name="stats", bufs=2) as stats_pool,
        tc.tile_pool(name="fc", bufs=1) as fc_pool,
        tc.tile_pool(name="sc", bufs=2, space="PSUM") as sc_pool,
        tc.tile_pool(name="ps", bufs=2, space="PSUM") as ps_pool,
        tc.tile_pool(name="acc", bufs=2, space="PSUM") as acc_pool,
    ):
        # ---- critical-path DMAs first: QKV weights, then batch-0 x^T ----
        wqkv_t = []
        for i in range(NDT):
            t = params.tile([128, 301], FR, name=f"wqkv{i}", tag=f"wqkv{i}")
            nc.sync.dma_start(t[:], aps["wqkv"][i * 128:(i + 1) * 128, :])
            wqkv_t.append(t)
        xtt = {}

        def load_xt(b):
            xtt[b] = []
            for i in range(NDT):
                t = xt_pool.tile([128, S], FR, name=f"xt{b}_{i}", tag="xt")
                nc.sync.dma_start(t[:], aps["xt"][b, i * 128:(i + 1) * 128, :])
                xtt[b].append(t)

        load_xt(0)

        # ---- remaining parameters (overlap with early compute) ----
        adp_sb = params.tile([H + 1, DP], FR, name="adp", tag="adp")
        nc.sync.dma_start(adp_sb[:], aps["adp"][:])
        d1w_t = []
        for i in range(NDT):
            t = params.tile([128, DP], FR, name=f"d1w{i}", tag=f"d1w{i}")
            nc.sync.dma_start(t[:], aps["d1w"][i * 128:(i + 1) * 128, :])
            d1w_t.append(t)
        d1b_sb = params.tile([128, 4], FP, name="d1b", tag="d1b")
        nc.sync.dma_start(d1b_sb[:], aps["d1b"][:])
        fc1w_t = []
        for i in range(8):
            t = params.tile([128, 512], FR, name=f"fc1w{i}", tag=f"fc1w{i}")
            nc.sync.dma_start(t[:], aps["fc1w"][i * 128:(i + 1) * 128, :])
            fc1w_t.append(t)
        fc1b_sb = params.tile([128, 4], FP, name="fc1b", tag="fc1b")
        nc.sync.dma_start(fc1b_sb[:], aps["fc1b"][:])
        fc2w_t = []
        for i in range(4):
            t = params.tile([128, 256], FR, name=f"fc2w{i}", tag=f"fc2w{i}")
            nc.sync.dma_start(t[:], aps["fc2w"][i * 128:(i + 1) * 128, :])
            fc2w_t.append(t)
        fc2b_sb = params.tile([128, 2], FP, name="fc2b", tag="fc2b")
        nc.sync.dma_start(fc2b_sb[:], aps["fc2b"][:])
        ident_sb = params.tile([128, 128], FP, name="ident", tag="ident")
        nc.sync.dma_start(ident_sb[:], aps["ident"][:])

        ones_sb = params.tile([1, H + 1], FP, name="ones", tag="ones")
        nc.vector.memset(ones_sb[:], 1.0)
        ones_fr = params.tile([1, H + 1], FR, name="onesfr", tag="onesfr")
        nc.vector.tensor_copy(ones_fr[:], ones_sb[:])
        onescol = params.tile([128, 1], FP, name="onescol", tag="onescol")
        nc.vector.memset(onescol[:], 1.0)
        zeros_sb = params.tile([128, 512], FP, name="zeros", tag="zeros")
        nc.vector.memset(zeros_sb[:], 0.0)

        # pooled feature vector, [128, BL] per p-tile: 0-3 max part, 4-7 sums
        pooled = [params.tile([128, BL], FP, name=f"pooled{p}",
                              tag=f"pooled{p}") for p in range(8)]

        state = {}

        def qkv_phase(b):
            qt = qkv_pool.tile([H, S], FR, name=f"qt{b}", tag="qt")
            kt = qkv_pool.tile([H, S], FR, name=f"kt{b}", tag="kt")
            # vt has 101 rows: V rows permuted with a spare row at OC (the
            # packed V weight block carries a zero column there)
            vt = qkv_pool.tile([H + 1, S], FP, name=f"vt{b}", tag="vt")
            proj = ((qt, 0, H), (kt, H, 2 * H), (vt, 2 * H, 3 * H + 1))
            for qb in range(NQB):
                sl = slice(qb * 512, (qb + 1) * 512)
                for pi, (dst, c0, c1) in enumerate(proj):
                    ps = ps_pool.tile([c1 - c0, 512], FP,
                                      name=f"qkvp{b}_{qb}_{pi}", tag="ps")
                    for di in range(NDT):
                        nc.tensor.matmul(
                            ps[:], wqkv_t[di][:, c0:c1],
                            xtt[b][di][:, sl],
                            start=(di == 0), stop=(di == NDT - 1))
                    nc.vector.tensor_copy(dst[:, sl], ps[:])

            # V^T -> V tiles; overwrite the spare column with ones
            vaug = []
            for s in range(NKT):
                pv = ps_pool.tile([128, H + 1], FP, name=f"vtp{b}_{s}",
                                  tag="ps")
                nc.tensor.transpose(pv[:], vt[:, s * 128:(s + 1) * 128],
                                    ident_sb[0:H + 1, 0:H + 1])
                va = vaug_pool.tile([128, H + 1], FR, name=f"vaug{b}_{s}",
                                    tag="vaug")
                nc.vector.tensor_copy(va[:], pv[:])
                nc.vector.tensor_copy(va[:, OC:OC + 1], onescol[:])
                vaug.append(va)

            bmax = [stats_pool.tile([128, NQB], FP, name=f"bmax{b}_{f}",
                                    tag=f"bmax{f}") for f in range(4)]
            bsum = [stats_pool.tile([128, NQB], FP, name=f"bsum{b}_{f}",
                                    tag=f"bsum{f}") for f in range(4)]
            state[b] = dict(qt=qt, kt=kt, vaug=vaug, bmax=bmax, bsum=bsum)

        def attention_block(b, qb):
            st = state[b]
            sl = slice(qb * 512, (qb + 1) * 512)
            avt = acc_pool.tile([H + 1, 512], FP, name=f"avt{b}_{qb}",
                                tag="avt")
            # k-tiles processed in pairs: two 512-wide score matmuls land in
            # one 2-bank PSUM region, one exp covers both (halves ACT
            # per-op overhead), then two AV accumulation matmuls.
            for kp in range(NKT // 2):
                k0, k1 = 2 * kp, 2 * kp + 1
                sc = sc_pool.tile([128, 1024], FP, name=f"sc{b}_{qb}_{kp}",
                                  tag="sc")
                nc.tensor.matmul(sc[:, 0:512],
                                 st["kt"][:, k0 * 128:(k0 + 1) * 128],
                                 st["qt"][:, sl], start=True, stop=True)
                nc.tensor.matmul(sc[:, 512:1024],
                                 st["kt"][:, k1 * 128:(k1 + 1) * 128],
                                 st["qt"][:, sl], start=True, stop=True)
                pt = pt_pool.tile([128, 1024], FR, name=f"pt{b}_{qb}_{kp}",
                                  tag="pt")
                nc.scalar.activation(pt[:], sc[:], Exp, scale=SCALE)
                nc.tensor.matmul(avt[:], st["vaug"][k0][:], pt[:, 0:512],
                                 start=(kp == 0), stop=False)
                nc.tensor.matmul(avt[:], st["vaug"][k1][:], pt[:, 512:1024],
                                 start=False, stop=(kp == NKT // 2 - 1))
            return avt

        def finish_a(b, qb, avt):
            # softmax normalization: 1/denom as exp(-ln(denom)) on ACT
            # (Ln and Exp share one activation table set; DVE's exact
            # reciprocal is 8 cycles/element and far slower).  Emitted right
            # after this block's attention so the two ops sit *ahead* of the
            # next block's exps in the ACT queue.
            lnd = attn_pool.tile([1, 512], FP, name=f"lnd{b}_{qb}", tag="lnd")
            nc.scalar.activation(lnd[:], avt[OC:OC + 1, :],
                                 mybir.ActivationFunctionType.Ln)
            recip_r = attn_pool.tile([1, 512], FR, name=f"recr{b}_{qb}",
                                     tag="recr")
            nc.scalar.activation(recip_r[:], lnd[:], Exp, scale=-1.0)
            return recip_r

        def finish_b(b, qb, avt, recip_r):
            st = state[b]
            bcp = ps_pool.tile([H + 1, 512], FP, name=f"bcp{b}_{qb}", tag="ps")
            nc.tensor.matmul(bcp[:], ones_fr[:], recip_r[:],
                             start=True, stop=True)
            bc = attn_pool.tile([H + 1, 512], FP, name=f"bc{b}_{qb}", tag="bc")
            nc.vector.tensor_copy(bc[:], bcp[:])
            avtn = attn_pool.tile([H + 1, 512], FR, name=f"avtn{b}_{qb}",
                                  tag="avtn")
            nc.vector.tensor_tensor(avtn[:], avt[:], bc[:],
                                    op=mybir.AluOpType.mult)

            # attention output projection (head-folded, row-permuted AD')
            ao = []
            for e in range(NDT):
                pa = ps_pool.tile([128, 512], FP, name=f"ao{b}_{qb}_{e}",
                                  tag="ps")
                nc.tensor.matmul(pa[:], adp_sb[:, e * 128:(e + 1) * 128],
                                 avtn[:], start=True, stop=True)
                at = ao_pool.tile([128, 512], FR, name=f"aot{b}_{qb}_{e}",
                                  tag="ao")
                nc.vector.tensor_copy(at[:], pa[:])
                ao.append(at)

            # d1 + bias + relu; row-sums accumulated for mean pooling
            for f in range(4):
                ph = ps_pool.tile([128, 512], FP, name=f"h1p{b}_{qb}_{f}",
                                  tag="ps")
                for e in range(NDT):
                    nc.tensor.matmul(ph[:],
                                     d1w_t[e][:, f * 128:(f + 1) * 128],
                                     ao[e][:], start=(e == 0),
                                     stop=(e == NDT - 1))
                h1 = h1_pool.tile([128, 512], FP, name=f"h1{b}_{qb}_{f}",
                                  tag="h1")
                # relu(ph + bias) with fused row-sum, on DVE (ACT is busy
                # with exp; this keeps d1 PSUM evacuation off its queue)
                nc.vector.scalar_tensor_tensor(
                    h1[:], ph[:], d1b_sb[:, f:f + 1], zeros_sb[:],
                    op0=mybir.AluOpType.add, op1=mybir.AluOpType.max,
                    accum_out=st["bsum"][f][:, qb:qb + 1])
                nc.vector.reduce_max(st["bmax"][f][:, qb:qb + 1], h1[:],
                                     axis=X)

            if qb == NQB - 1:
                for f in range(4):
                    nc.vector.reduce_max(pooled[f][:, b:b + 1],
                                         st["bmax"][f][:], axis=X)
                    nc.vector.reduce_sum(pooled[4 + f][:, b:b + 1],
                                         st["bsum"][f][:], axis=X)

        # ---- pipelined emission over all (batch, q-block) pairs ----
        prev = None
        for b in range(BL):
            if b > 0:
                load_xt(b)
            qkv_phase(b)
            for qb in range(NQB):
                avt = attention_block(b, qb)
                rec = finish_a(b, qb, avt)
                if prev is not None:
                    finish_b(*prev)
                prev = (b, qb, avt, rec)
        finish_b(*prev)

        # rounded copies of pooled for the f32r fc matmuls
        pooled_r = []
        for p in range(8):
            t = params.tile([128, BL], FR, name=f"pooledr{p}",
                            tag=f"pooledr{p}")
            nc.vector.tensor_copy(t[:], pooled[p][:])
            pooled_r.append(t)

        # ---- fc head (both batches together; N = BL columns) ----
        pf1 = ps_pool.tile([BL, 512], FP, name="fc1ps", tag="ps")
        for p in range(8):
            nc.tensor.matmul(pf1[:], pooled_r[p][:], fc1w_t[p][:],
                             start=(p == 0), stop=(p == 7))
        o1 = fc_pool.tile([BL, 512], FP, name="o1", tag="o1")
        nc.vector.tensor_copy(o1[:], pf1[:])
        h1fc = []
        for g in range(4):
            pt_ = ps_pool.tile([128, BL], FP, name=f"fct{g}", tag="ps")
            nc.tensor.transpose(pt_[:], o1[:, g * 128:(g + 1) * 128],
                                ident_sb[0:BL, 0:BL])
            hg = fc_pool.tile([128, BL], FR, name=f"h1fc{g}", tag=f"h1fc{g}")
            nc.scalar.activation(hg[:], pt_[:], Relu, bias=fc1b_sb[:, g:g + 1])
            h1fc.append(hg)

        pf2 = ps_pool.tile([BL, 256], FP, name="fc2ps", tag="ps")
        for g in range(4):
            nc.tensor.matmul(pf2[:], h1fc[g][:], fc2w_t[g][:],
                             start=(g == 0), stop=(g == 3))
        o2 = fc_pool.tile([BL, 256], FP, name="o2", tag="o2")
        nc.vector.tensor_copy(o2[:], pf2[:])
        for o in range(2):
            pt2 = ps_pool.tile([128, BL], FP, name=f"fct2{o}", tag="ps")
            nc.tensor.transpose(pt2[:], o2[:, o * 128:(o + 1) * 128],
                                ident_sb[0:BL, 0:BL])
            ot = fc_pool.tile([128, BL], FP, name=f"ot{o}", tag=f"ot{o}")
            nc.scalar.activation(ot[:], pt2[:], Relu, bias=fc2b_sb[:, o:o + 1])
            nc.sync.dma_start(aps["out"][o * 128:(o + 1) * 128, :], ot[:])


def _pin_activation_table(arch="gen3"):
    """All activations used here (exp, ln, relu) live in one table set
    (natural_log_exp_and_others).  The table-load inserter picks the first
    set containing each function, which ping-pongs exp<->ln sets and costs
    a ~1.3us ACT_TABLE_LOAD per switch.  Empty every other set in the cached
    table dict (indices preserved) so exactly one load is ever emitted."""
    import concourse.hw_specs as hw_specs
    tables = hw_specs.get_activation_tables(arch)
    for name, funcs in tables.items():
        if name != "natural_log_exp_and_others":
            funcs.clear()


def build_nc():
    # Bacc (not raw Bass): finalize() runs move_matmul_waits_to_ldweights +
    # generate_event_semaphores, legalizing multi-wait instructions for the
    # 1-wait-slot self-loading fp32/f32r matmul struct.
    nc = bacc.Bacc("TRN2", target_bir_lowering=False, debug=False,
                   num_devices=NCORES)
    _pin_activation_table(nc.m.arch)
    aps = {
        "xt": nc.dram_tensor("xt", [BL, DP, S], FR, kind="ExternalInput").ap(),
        "wqkv": nc.dram_tensor("wqkv", [DP, 3 * H + 1], FR,
                               kind="ExternalInput").ap(),
        "adp": nc.dram_tensor("adp", [H + 1, DP], FR,
                              kind="ExternalInput").ap(),
        "d1w": nc.dram_tensor("d1w", [DP, DP], FR, kind="ExternalInput").ap(),
        "d1b": nc.dram_tensor("d1b", [128, 4], FP, kind="ExternalInput").ap(),
        "fc1w": nc.dram_tensor("fc1w", [1024, 512], FR,
                               kind="ExternalInput").ap(),
        "fc1b": nc.dram_tensor("fc1b", [128, 4], FP,
                               kind="ExternalInput").ap(),
        "fc2w": nc.dram_tensor("fc2w", [512, 256], FR,
                               kind="ExternalInput").ap(),
        "fc2b": nc.dram_tensor("fc2b", [128, 2], FP,
                               kind="ExternalInput").ap(),
        "ident": nc.dram_tensor("ident", [128, 128], FP,
                                kind="ExternalInput").ap(),
        "out": nc.dram_tensor("out", [256, BL], FP, kind="ExternalOutput").ap(),
    }
    with tile.TileContext(nc) as tc:
        build_kernel_body(tc, aps)
    nc.finalize()
    return nc


_NC_CACHE = None


def get_nc():
    global _NC_CACHE
    if _NC_CACHE is None:
        _NC_CACHE = build_nc()
    return _NC_CACHE


def prep_inputs(x, attn_kernel, attn_dense, d1_w, d1_b, fc1_w, fc1_b,
                fc2_w, fc2_b):
    """Host-side layout prep. Returns per-core input maps."""
    x = np.ascontiguousarray(np.asarray(x, np.float32))
    attn_kernel = np.asarray(attn_kernel, np.float32)
    attn_dense = np.asarray(attn_dense, np.float32)
    d1_w = np.asarray(d1_w, np.float32)
    d1_b = np.asarray(d1_b, np.float32)
    fc1_w = np.asarray(fc1_w, np.float32)
    fc1_b = np.asarray(fc1_b, np.float32)
    fc2_w = np.asarray(fc2_w, np.float32)
    fc2_b = np.asarray(fc2_b, np.float32)

    xt = np.zeros((B, DP, S), np.float32)
    xt[:, :D, :] = x.transpose(0, 2, 1)

    wqkv = np.zeros((DP, 3 * H + 1), np.float32)
    wqkv[:D, 0:H] = attn_kernel[0]
    wqkv[:D, H:2 * H] = attn_kernel[1]
    # V block: 101 columns with a zero column at OC (ones slot), so V^T comes
    # out of the projection already in the permuted 101-row layout
    wqkv[:D, 2 * H:2 * H + OC] = attn_kernel[2][:, 0:OC]
    wqkv[:D, 2 * H + OC + 1:3 * H + 1] = attn_kernel[2][:, OC:H]

    ad_folded = (attn_dense[0:H] + attn_dense[H:2 * H]
                 + attn_dense[2 * H:3 * H] + attn_dense[3 * H:4 * H])
    # rows permuted to match the V_aug column order (ones column at OC)
    adp = np.zeros((H + 1, DP), np.float32)
    adp[0:OC, :D] = ad_folded[0:OC]
    adp[OC + 1:H + 1, :D] = ad_folded[OC:H]

    d1w = np.zeros((DP, DP), np.float32)
    d1w[:D, :D] = d1_w
    d1b = np.zeros((DP,), np.float32)
    d1b[:D] = d1_b
    d1b_t = np.ascontiguousarray(d1b.reshape(4, 128).T)

    fc1w = np.zeros((2 * DP, 512), np.float32)
    fc1w[0:D] = fc1_w[0:D]
    fc1w[DP:DP + D] = fc1_w[D:2 * D] * np.float32(1.0 / S)
    fc1b_t = np.ascontiguousarray(fc1_b.reshape(4, 128).T)
    fc2b_t = np.ascontiguousarray(fc2_b.reshape(2, 128).T)

    ident = np.eye(128, dtype=np.float32)

    shared = dict(wqkv=wqkv, adp=adp, d1w=d1w, d1b=d1b_t, fc1w=fc1w,
                  fc1b=fc1b_t, fc2w=np.ascontiguousarray(fc2_w),
                  fc2b=fc2b_t, ident=ident)
    in_maps = []
    for c in range(NCORES):
        m = dict(shared)
        m["xt"] = np.ascontiguousarray(xt[c * BL:(c + 1) * BL])
        in_maps.append(m)
    return in_maps


def kernel(**inputs):
    in_maps = prep_inputs(**inputs)
    nc = get_nc()
    res = run_bass_kernel_spmd(nc, in_maps, core_ids=list(range(NCORES)))
    outs = [res.results[c]["out"] for c in range(NCORES)]
    return np.ascontiguousarray(
        np.concatenate([o.T for o in outs], axis=0).astype(np.float32))
